# revision 17
# baseline (speedup 1.0000x reference)
"""MLA attention (DeepSeek-style) on 8 TRN2 NeuronCores.

Sharding: heads are tensor-parallel (2 heads/core; output channel blocks are
per-head independent in this formulation, so no output all-reduce). The shared
latent projections (c_q, c_kv, roped k_r) are token-parallel (512 tokens/core)
and exchanged with two AllGathers (c_q first, so per-head q prep overlaps the
second gather).

Formulation: instead of the absorbed k_eff = W_uq_h @ W_uk_h (which makes the
logits contraction 512-deep), attention uses the direct per-head factors
q_h = c_q @ W_uq_h and k_h = c_kv @ W_uk_h, so each logits chunk is one
128-deep matmul plus one 64-deep rope matmul. On the value side, the absorbed
v_eff = (W_uv.T @ W_o.T)_h is pre-applied per (batch, head) as
vt = c_kv @ v_eff_h, so AV is one 128-deep matmul per chunk and the output
projection disappears (y = attn @ vt directly). This halves PE work vs the
fully-absorbed layout.

Precision: bf16 throughout (same PE speed as f32r, half the DMA/SBUF/
collective bytes); PSUM accumulation is fp32.

Attention runs transposed, logitsT[s, t]: QK^T and AV need no transposes;
softmax needs no max subtraction (logits are O(1)) and column sums come from
an appended ones-row matmul; the host divides by the gathered sums.
"""

import math

import numpy as np

B, T, C = 4, 1024, 2048
NH, HS = 16, 128
NLQ = NLKV = 512
DHR = 64
NCORES = 8
HPC = NH // NCORES          # heads per core
TOK = (B * T) // NCORES     # tokens per core (half a batch)
SCALE = 1.0 / math.sqrt(HS + DHR)

# AllGather shard layouts. Every packed tensor's shard is exactly its SBUF
# tile layout [128, X] row-major, so pack and load DMAs are contiguous.
AG1_W = 4 * TOK                 # c_qT_own   [128, (q, t)]
COL_CKVT = 0                    # c_kvT_own  [128, (kc, s)]
COL_KR = 4 * TOK                # k_rT_own   [64, t] in a 512-col block
AG2_W = 4 * TOK + TOK           # 2560

_cache = {}


def _build(loop_k=None, sim_single=False):
    """Build the SPMD kernel. loop_k: if set, wrap phase 2 (attention) in a
    For_i(0, loop_k) hardware loop — used only for timing amplification.
    sim_single: single-core no-collective variant (gathers fed as inputs)
    for TimelineSim cost-model analysis."""
    import contextlib

    import concourse.mybir as mybir
    import concourse.tile as tile
    from concourse import bacc

    f32 = mybir.dt.float32
    f32r = mybir.dt.float32r
    bf16 = mybir.dt.bfloat16
    Exp = mybir.ActivationFunctionType.Exp
    Copy = mybir.ActivationFunctionType.Copy
    mult = mybir.AluOpType.mult
    add = mybir.AluOpType.add

    nc = bacc.Bacc(trn_type="TRN2", num_devices=1 if sim_single else NCORES)
    P = nc.declare_dram_parameter

    CC = C // 128  # 16 c-chunks

    # all weight/input params arrive pre-arranged in SBUF slab layout
    # [128, n_chunks * W] (host does the transform — contiguous DMAs here)
    xT = P("xT", [128, CC * TOK], bf16, isOutput=False)
    wdqT = P("wdqT", [128, CC * NLQ], bf16, isOutput=False)
    wdkvT = P("wdkvT", [128, CC * NLKV], bf16, isOutput=False)
    wkr2T = P("wkr2T", [128, CC * 2 * DHR], bf16, isOutput=False)
    wuq = P("wuq", [128, HPC * 4 * HS], bf16, isOutput=False)
    wukT = P("wukT", [128, HPC * 4 * HS], bf16, isOutput=False)
    wqr2T = P("wqr2T", [128, HPC * 4 * 2 * DHR], bf16, isOutput=False)
    wo2T = P("wo2T", [128, CC * HPC * HS], bf16, isOutput=False)
    wuv = P("wuv", [128, CC * NLKV], bf16, isOutput=False)
    cos2 = P("cos2", [DHR, T], f32, isOutput=False)
    sin2 = P("sin2", [DHR, T], f32, isOutput=False)
    cos2o = P("cos2o", [DHR, TOK], f32, isOutput=False)
    sin2o = P("sin2o", [DHR, TOK], f32, isOutput=False)
    maskp = P("maskp", [128, 128], bf16, isOutput=False)
    out = P("out", [HPC * HS, B * T], f32, isOutput=True)
    out2 = P("out2", [HPC, B * T], f32, isOutput=True)
    ag1_p = ag2_p = None
    if sim_single:
        ag1_p = P("ag1_p", [NCORES * 128, AG1_W], bf16, isOutput=False)
        ag2_p = P("ag2_p", [NCORES * 128, AG2_W], bf16, isOutput=False)

    with tile.TileContext(nc) as tc:
        with (
            tc.tile_pool(name="pres", bufs=1) as pres,
            tc.tile_pool(name="dram", bufs=1, space="DRAM") as dram,
            tc.tile_pool(name="ps_work", bufs=4, space="PSUM") as ps_work,
            tc.tile_pool(name="ps_av", bufs=2, space="PSUM") as ps_av,
            tc.tile_pool(name="ps_sums", bufs=2, space="PSUM") as ps_sums,
        ):
            # ---------- resident small tensors ----------
            wuq_sb = pres.tile([128, HPC * 4 * HS], bf16, tag="wuq")
            wukT_sb = pres.tile([128, HPC * 4 * HS], bf16, tag="wukT")
            wqr2T_sb = pres.tile([128, HPC * 4 * 2 * DHR], bf16, tag="wqr2T")
            cos2_sb = pres.tile([DHR, T], f32, tag="cos2")
            sin2_sb = pres.tile([DHR, T], f32, tag="sin2")
            cos2o_sb = pres.tile([DHR, TOK], f32, tag="cos2o")
            sin2o_sb = pres.tile([DHR, TOK], f32, tag="sin2o")
            v_eff_sb = pres.tile([128, 4 * HPC * HS], bf16, tag="v_eff")
            ones_sb = pres.tile([128, 1], f32, tag="ones")
            ones_r = pres.tile([128, 1], bf16, tag="ones_r")
            mask_r = pres.tile([128, 128], bf16, tag="mask_r")

            nc.sync.dma_start(wuq_sb[:], wuq[:, :])
            nc.sync.dma_start(wukT_sb[:], wukT[:, :])
            nc.sync.dma_start(wqr2T_sb[:], wqr2T[:, :])
            nc.sync.dma_start(cos2_sb[:], cos2[:])
            nc.sync.dma_start(sin2_sb[:], sin2[:])
            nc.sync.dma_start(cos2o_sb[:], cos2o[:])
            nc.sync.dma_start(sin2o_sb[:], sin2o[:])

            nc.gpsimd.memset(ones_sb[:], 1.0)
            nc.vector.tensor_copy(ones_r[:], ones_sb[:])
            # causal mask for a diagonal 128-col block: keep where s_i <= t_y
            nc.sync.dma_start(mask_r[:], maskp[:])

            # DRAM bounce buffers for the AllGathers
            agin1 = dram.tile([128, AG1_W], bf16)
            agin2 = dram.tile([128, AG2_W], bf16)
            agout1 = ag1_p if sim_single else dram.tile(
                [NCORES * 128, AG1_W], bf16, addr_space="Shared")
            agout2 = ag2_p if sim_single else dram.tile(
                [NCORES * 128, AG2_W], bf16, addr_space="Shared")

            # ---------- phase 0a: local preprocessing ----------
            pw = tc.alloc_tile_pool(name="pw", bufs=1)
            wuv_sb = pw.tile([128, CC * NLKV], bf16, tag="wuv")
            wo2T_sb = pw.tile([128, CC * HPC * HS], bf16, tag="wo2T")
            with tc.tile_pool(name="p0", bufs=1) as p0:
                xT_sb = p0.tile([128, CC * TOK], bf16, tag="xT")
                wdqT_sb = p0.tile([128, CC * NLQ], bf16, tag="wdqT")
                wdkvT_sb = p0.tile([128, CC * NLKV], bf16, tag="wdkvT")
                wkr2T_sb = p0.tile([128, CC * 2 * DHR], bf16, tag="wkr2T")
                cqT_own = p0.tile([128, 4 * TOK], bf16, tag="cqT_own")
                ckvT_own = p0.tile([128, 4 * TOK], bf16, tag="ckvT_own")
                krT_own = p0.tile([DHR, TOK], bf16, tag="krT_own")
                rtmp = p0.tile([DHR, 2 * TOK], f32, tag="rtmp")

                for qr_ in range(8):
                    csl = slice(qr_ * 2 * TOK, (qr_ + 1) * 2 * TOK)
                    nc.sync.dma_start(xT_sb[:, csl], xT[:, csl])
                    wsl = slice(qr_ * 2 * NLQ, (qr_ + 1) * 2 * NLQ)
                    nc.sync.dma_start(wdqT_sb[:, wsl], wdqT[:, wsl])
                    nc.sync.dma_start(wdkvT_sb[:, wsl], wdkvT[:, wsl])
                    ksl = slice(qr_ * 4 * DHR, (qr_ + 1) * 4 * DHR)
                    nc.sync.dma_start(wkr2T_sb[:, ksl], wkr2T[:, ksl])
                # prefetch the absorbed-value weights behind the phase-0a
                # inputs so v_eff can start the moment the PE frees up
                nc.sync.dma_start(wuv_sb[:], wuv[:, :])
                nc.sync.dma_start(wo2T_sb[:], wo2T[:, :])

                # c_qT_own[q, t] then its AllGather right away
                for qt in range(4):
                    pq = ps_work.tile([128, TOK], f32, tag="work")
                    for cc in range(CC):
                        nc.tensor.matmul(
                            pq[:],
                            wdqT_sb[:, cc * NLQ + qt * 128: cc * NLQ + (qt + 1) * 128],
                            xT_sb[:, cc * TOK:(cc + 1) * TOK],
                            start=(cc == 0), stop=(cc == CC - 1))
                    nc.vector.tensor_copy(cqT_own[:, qt * TOK:(qt + 1) * TOK], pq[:])
                nc.gpsimd.dma_start(agin1[:, :], cqT_own[:])
                if not sim_single:
                    nc.gpsimd.collective_compute(
                        "AllGather", mybir.AluOpType.bypass,
                        replica_groups=[list(range(NCORES))],
                        ins=[agin1.opt()], outs=[agout1.opt()])

                for kt in range(4):
                    pk = ps_work.tile([128, TOK], f32, tag="work")
                    for cc in range(CC):
                        nc.tensor.matmul(
                            pk[:],
                            wdkvT_sb[:, cc * NLKV + kt * 128: cc * NLKV + (kt + 1) * 128],
                            xT_sb[:, cc * TOK:(cc + 1) * TOK],
                            start=(cc == 0), stop=(cc == CC - 1))
                    nc.vector.tensor_copy(ckvT_own[:, kt * TOK:(kt + 1) * TOK], pk[:])
                nc.gpsimd.dma_start(agin2[:, COL_CKVT:COL_CKVT + 4 * TOK],
                                    ckvT_own[:])
                # k_r (roped): rows 0..63 plain, 64..127 pair-swapped copy
                pr = ps_work.tile([128, TOK], f32, tag="work")
                for cc in range(CC):
                    nc.tensor.matmul(
                        pr[:],
                        wkr2T_sb[:, cc * 2 * DHR:(cc + 1) * 2 * DHR],
                        xT_sb[:, cc * TOK:(cc + 1) * TOK],
                        start=(cc == 0), stop=(cc == CC - 1))
                nc.vector.tensor_tensor(rtmp[:, :TOK], pr[:DHR, :], cos2o_sb[:], mult)
                nc.vector.tensor_tensor(rtmp[:, TOK:], pr[DHR:, :], sin2o_sb[:], mult)
                nc.vector.tensor_tensor(krT_own[:], rtmp[:, :TOK], rtmp[:, TOK:], add)
                nc.gpsimd.dma_start(agin2[:DHR, COL_KR:COL_KR + TOK], krT_own[:])

            if not sim_single:
                nc.gpsimd.collective_compute(
                    "AllGather", mybir.AluOpType.bypass,
                    replica_groups=[list(range(NCORES))],
                    ins=[agin2.opt()], outs=[agout2.opt()])

            # ---------- phase 0b: absorbed value weights ----------
            # v_eff[k', d2] (d2 = both heads' HS), bf16 for the vt matmuls
            for kt in range(4):
                pv = ps_work.tile([128, HPC * HS], f32, tag="work")
                for cc in range(CC):
                    nc.tensor.matmul(
                        pv[:],
                        wuv_sb[:, cc * NLKV + kt * 128: cc * NLKV + (kt + 1) * 128],
                        wo2T_sb[:, cc * HPC * HS:(cc + 1) * HPC * HS],
                        start=(cc == 0), stop=(cc == CC - 1))
                nc.scalar.activation(
                    v_eff_sb[:, kt * HPC * HS:(kt + 1) * HPC * HS], pv[:], Copy)
            pw.release()

            # ---------- phase 2: attention per (batch, head) ----------
            with (
                tc.tile_pool(name="pl", bufs=1) as pl,
                tc.tile_pool(name="pb", bufs=2) as pb,
                tc.tile_pool(name="ph", bufs=2) as ph,
                tc.tile_pool(name="pex", bufs=16) as pex,
                tc.tile_pool(name="py", bufs=2) as py,
            ):
                # preload every batch's gathered tensors once (the loop body
                # then runs DMA-free on the input side). Fully contiguous
                # per-rank loads: the shard layout in the bounce equals the
                # SBUF layout.
                cqT_B, ckvT_B, krT_B = [], [], []
                ag1r = agout1.ap() if sim_single else agout1[:]
                ag2r = agout2.ap() if sim_single else agout2[:]
                for b in range(B):
                    cqT_B.append(pl.tile([128, 4 * T], bf16, tag=f"cqT_{b}"))
                    ckvT_B.append(pl.tile([128, 4 * T], bf16, tag=f"ckvT_{b}"))
                    krT_B.append(pl.tile([DHR, T], bf16, tag=f"krT_{b}"))
                    for half in range(2):
                        r = 2 * b + half
                        nc.sync.dma_start(
                            cqT_B[b][:, half * 4 * TOK:(half + 1) * 4 * TOK],
                            ag1r[r * 128:(r + 1) * 128, :])
                        nc.sync.dma_start(
                            ckvT_B[b][:, half * 4 * TOK:(half + 1) * 4 * TOK],
                            ag2r[r * 128:(r + 1) * 128, COL_CKVT:COL_CKVT + 4 * TOK])
                        nc.sync.dma_start(
                            krT_B[b][:, half * TOK:(half + 1) * TOK],
                            ag2r[r * 128: r * 128 + DHR, COL_KR:COL_KR + TOK])

                with (tc.For_i(0, loop_k, 1) if loop_k
                      else contextlib.nullcontext()):
                    for b in range(B):
                        cqT_b = cqT_B[b]
                        ckvT_b = ckvT_B[b]
                        krT_b = krT_B[b]

                    # vt[s, d2] = c_kv @ v_eff for both heads, s-major slabs
                    # (8 s-chunks x 256 cols) — the AV lhsT
                    vt_b = pb.tile([128, 8 * HPC * HS], bf16, tag="vt_b")
                    for sc2 in range(4):
                        pvt = ps_work.tile([128, 512], f32, tag="work")
                        for half in range(2):
                            sc = 2 * sc2 + half
                            sb2 = (sc // 4) * 2048 + (sc % 4) * 128
                            for kc in range(4):
                                nc.tensor.matmul(
                                    pvt[:, half * 256:(half + 1) * 256],
                                    ckvT_b[:, sb2 + kc * 512: sb2 + kc * 512 + 128],
                                    v_eff_sb[:, kc * 256:(kc + 1) * 256],
                                    start=(kc == 0), stop=(kc == 3))
                        if sc2 % 2 == 0:
                            nc.scalar.activation(
                                vt_b[:, sc2 * 512:(sc2 + 1) * 512], pvt[:], Copy)
                        else:
                            nc.vector.tensor_copy(
                                vt_b[:, sc2 * 512:(sc2 + 1) * 512], pvt[:])

                    def prep(hh):
                        # k_hT[d, s], q_hT[d, t], roped q_rT[d, t] per head
                        khT_sb = ph.tile([128, T], bf16, tag="khT")
                        qhT_sb = ph.tile([128, T], bf16, tag="qhT")
                        for th in range(2):
                            pk = ps_work.tile([128, 512], f32, tag="work")
                            for kc in range(4):
                                nc.tensor.matmul(
                                    pk[:],
                                    wukT_sb[:, (hh * 4 + kc) * 128:(hh * 4 + kc + 1) * 128],
                                    ckvT_b[:, th * 2048 + kc * 512:
                                           th * 2048 + (kc + 1) * 512],
                                    start=(kc == 0), stop=(kc == 3))
                            if th == 0:
                                nc.scalar.activation(
                                    khT_sb[:, th * 512:(th + 1) * 512], pk[:],
                                    Copy)
                            else:
                                nc.vector.tensor_copy(
                                    khT_sb[:, th * 512:(th + 1) * 512], pk[:])
                        for th in range(2):
                            pq = ps_work.tile([128, 512], f32, tag="work")
                            for qc in range(4):
                                nc.tensor.matmul(
                                    pq[:],
                                    wuq_sb[:, (hh * 4 + qc) * 128:(hh * 4 + qc + 1) * 128],
                                    cqT_b[:, th * 2048 + qc * 512:
                                          th * 2048 + (qc + 1) * 512],
                                    start=(qc == 0), stop=(qc == 3))
                            if th == 0:
                                nc.scalar.activation(
                                    qhT_sb[:, th * 512:(th + 1) * 512], pq[:],
                                    Copy)
                            else:
                                nc.vector.tensor_copy(
                                    qhT_sb[:, th * 512:(th + 1) * 512], pq[:])
                        qrT_sb = ph.tile([DHR, T], bf16, tag="qrT")
                        qrtmp = ph.tile([DHR, 2 * 512], f32, tag="qrtmp")
                        for ts2 in range(2):
                            pq = ps_work.tile([128, 512], f32, tag="work")
                            for qc in range(4):
                                nc.tensor.matmul(
                                    pq[:],
                                    wqr2T_sb[:, (hh * 4 + qc) * 128:(hh * 4 + qc + 1) * 128],
                                    cqT_b[:, ts2 * 2048 + qc * 512:
                                          ts2 * 2048 + (qc + 1) * 512],
                                    start=(qc == 0), stop=(qc == 3))
                            sl = slice(ts2 * 512, (ts2 + 1) * 512)
                            nc.vector.tensor_tensor(
                                qrtmp[:, :512], pq[:DHR, :], cos2_sb[:, sl], mult)
                            nc.vector.tensor_tensor(
                                qrtmp[:, 512:], pq[DHR:, :], sin2_sb[:, sl], mult)
                            nc.vector.tensor_tensor(
                                qrT_sb[:, sl], qrtmp[:, :512], qrtmp[:, 512:], add)
                        return khT_sb, qhT_sb, qrT_sb

                    def pass1(hh, ts2, tiles):
                        # all logits + exp for this t-span. Keeping the AV
                        # matmuls out of the logits->exp dependency chain lets
                        # the PE run ahead instead of stalling on ACT.
                        khT_sb, qhT_sb, qrT_sb = tiles
                        t0 = ts2 * 512
                        exs = []
                        for j in range(4 * ts2 + 4):
                            t_off = max(0, 128 * j - t0)
                            njt = 512 - t_off
                            tsl = slice(t0 + t_off, t0 + 512)
                            lg = ps_work.tile([128, 512], f32, tag="work")
                            nc.tensor.matmul(
                                lg[:, :njt],
                                khT_sb[:, j * 128:(j + 1) * 128],
                                qhT_sb[:, tsl],
                                start=True, stop=False)
                            nc.tensor.matmul(
                                lg[:, :njt],
                                krT_b[:, j * 128:(j + 1) * 128],
                                qrT_sb[:, tsl],
                                start=False, stop=True)
                            ex = pex.tile([128, 512], bf16, tag="ex",
                                          name=f"ex_{b}_{hh}_{ts2}_{j}")
                            nc.scalar.activation(ex[:, :njt], lg[:, :njt],
                                                 Exp, scale=SCALE)
                            if 128 * j >= t0:
                                nc.gpsimd.tensor_tensor(
                                    ex[:, :128], ex[:, :128], mask_r[:], mult)
                            exs.append((ex, t_off, njt))
                        return exs

                    def pass2(hh, ts2, exs, y2_sb, sums2_sb):
                        # AV + sums accumulation over all chunks; the host
                        # divides the unnormalized output by the sums
                        t0 = ts2 * 512
                        n_s = 4 * ts2 + 4
                        av_ps = ps_av.tile([128, 512], f32, tag="av",
                                           name=f"av_{b}_{hh}_{ts2}")
                        sums_ps = ps_sums.tile([1, 512], f32, tag="sums")
                        for j, (ex, t_off, njt) in enumerate(exs):
                            last = (j == n_s - 1)
                            nc.tensor.matmul(
                                av_ps[:, t_off:],
                                vt_b[:, j * 256 + hh * 128: j * 256 + (hh + 1) * 128],
                                ex[:, :njt],
                                start=(j == 0), stop=last)
                            nc.tensor.matmul(
                                sums_ps[:, t_off:], ones_r[:], ex[:, :njt],
                                start=(j == 0), stop=last)
                        nc.vector.tensor_copy(
                            sums2_sb[:, t0:t0 + 512], sums_ps[:])
                        nc.vector.tensor_copy(
                            y2_sb[:, t0:t0 + 512], av_ps[:])

                    # software pipeline: PE fills with the next head's prep
                    # while ACT drains the current head's exp chain
                    y2 = [None, None]
                    s2 = [None, None]
                    tiles0 = prep(0)
                    ex00 = pass1(0, 0, tiles0)
                    ex01 = pass1(0, 1, tiles0)
                    y2[0] = py.tile([128, 2 * 512], f32, tag="y2",
                                    name=f"y2_{b}_0")
                    s2[0] = py.tile([1, 2 * 512], f32, tag="sums2",
                                    name=f"s2_{b}_0")
                    pass2(0, 0, ex00, y2[0], s2[0])
                    tiles1 = prep(1)
                    pass2(0, 1, ex01, y2[0], s2[0])
                    nc.sync.dma_start(
                        out[0: HS, b * T: (b + 1) * T], y2[0][:])
                    nc.sync.dma_start(
                        out2[0: 1, b * T: (b + 1) * T], s2[0][:])
                    ex10 = pass1(1, 0, tiles1)
                    ex11 = pass1(1, 1, tiles1)
                    y2[1] = py.tile([128, 2 * 512], f32, tag="y2",
                                    name=f"y2_{b}_1")
                    s2[1] = py.tile([1, 2 * 512], f32, tag="sums2",
                                    name=f"s2_{b}_1")
                    pass2(1, 0, ex10, y2[1], s2[1])
                    pass2(1, 1, ex11, y2[1], s2[1])
                    nc.sync.dma_start(
                        out[HS: 2 * HS, b * T: (b + 1) * T], y2[1][:])
                    nc.sync.dma_start(
                        out2[1: 2, b * T: (b + 1) * T], s2[1][:])
    nc.compile()
    return nc


def _pairswap(w):
    idx = np.arange(w.shape[0]).reshape(-1, 2)[:, ::-1].reshape(-1)
    return w[idx]


def _slab(m, dtype=np.float32):
    """[n*128, W] row-major -> SBUF slab layout [128, n*W]."""
    n = m.shape[0] // 128
    return np.ascontiguousarray(
        m.reshape(n, 128, m.shape[1]).transpose(1, 0, 2).reshape(128, -1),
        dtype=dtype)


def _make_in_maps(x, W_dq, W_uq, W_dkv, W_uk, W_uv, W_o, W_qr, W_kr,
                  freqs_cos, freqs_sin):
    import ml_dtypes
    f4 = np.float32
    bf = ml_dtypes.bfloat16
    wdqT = _slab(W_dq.T, dtype=bf)
    wdkvT = _slab(W_dkv.T, dtype=bf)
    wkr2T = _slab(np.concatenate([W_kr.T, _pairswap(W_kr).T], axis=1),
                  dtype=bf)
    wuv = _slab(W_uv, dtype=bf)
    uq = W_uq.reshape(NLQ, NH, HS)
    uk = W_uk.reshape(NH, HS, NLKV)
    cos2 = np.repeat(freqs_cos.T, 2, axis=0).astype(f4)          # [DHR, T]
    sin_half = freqs_sin.T.astype(f4)                            # [DHR/2, T]
    sin2 = np.empty((DHR, T), dtype=f4)
    sin2[0::2] = -sin_half
    sin2[1::2] = sin_half

    in_maps = []
    for i in range(NCORES):
        b_own, half = divmod(i, 2)
        t0 = half * TOK
        heads = [HPC * i + hh for hh in range(HPC)]
        wuq_s = _slab(np.concatenate([uq[:, h, :] for h in heads], axis=0),
                      dtype=bf)
        wukT_s = _slab(np.concatenate([uk[h].T for h in heads], axis=0),
                       dtype=bf)
        wqr2T = _slab(np.concatenate([
            np.concatenate(
                [W_qr[h * DHR:(h + 1) * DHR].T,
                 _pairswap(W_qr[h * DHR:(h + 1) * DHR]).T], axis=1)
            for h in heads], axis=0), dtype=bf)
        wo2T = _slab(W_o[heads[0] * HS: (heads[-1] + 1) * HS, :].T, dtype=bf)
        in_maps.append({
            "xT": _slab(x[b_own, t0:t0 + TOK, :].T, dtype=bf),
            "wdqT": wdqT, "wdkvT": wdkvT, "wkr2T": wkr2T,
            "wuq": wuq_s, "wukT": wukT_s,
            "wqr2T": wqr2T, "wo2T": wo2T, "wuv": wuv,
            "cos2": cos2, "sin2": sin2,
            "cos2o": np.ascontiguousarray(cos2[:, t0:t0 + TOK]),
            "sin2o": np.ascontiguousarray(sin2[:, t0:t0 + TOK]),
            "maskp": np.triu(np.ones((128, 128))).astype(bf),
        })
    return in_maps


def _assemble(results):
    y = np.empty((B, T, C), dtype=np.float32)
    for i in range(NCORES):
        o = results[i]["out"]    # [HPC*HS, B*T] (unnormalized)
        s2 = results[i]["out2"]  # [HPC, B*T] softmax denominators
        for hh in range(HPC):
            h = HPC * i + hh
            blk = (o[hh * HS:(hh + 1) * HS, :] / s2[hh]).reshape(HS, B, T)
            y[:, :, h * HS:(h + 1) * HS] = blk.transpose(1, 2, 0)
    return y


def kernel(**inputs):
    from concourse import bass_utils
    if "nc" not in _cache:
        _cache["nc"] = _build()
    nc = _cache["nc"]
    in_maps = _make_in_maps(**{k: np.asarray(v) for k, v in inputs.items()})
    res = bass_utils.run_bass_kernel_spmd(nc, in_maps, core_ids=list(range(NCORES)))
    return _assemble(res.results)
